# revision 16
# baseline (speedup 1.0000x reference)
"""BatchHardTripletLoss on 8 TRN2 NeuronCores (Bass/Tile).

Data-parallel, SPMD-uniform strategy:
  - Host: sort rows by label, pad every class segment to SEG=1024 rows with
    far-away dummy rows (first coord DUMMY_VAL -> sq ~ 1e6, so dummies never
    win a hardest-negative).  Core i gets one 128-row anchor tile per class
    (tile index (i+k)%8 of class k), so local anchor tile k is class k on
    every core and one SPMD program has fully static slicing.
  - On chip: j (candidate rows) live on PSUM partitions, anchors on the free
    dim.  TensorE computes +2*E_jtile@A^T in bf16.  One native DVE
    scalar_tensor_tensor per range fuses everything:
        acc = (psum - sq_j) max/min acc        (sq_j is per-partition!)
    hardest-negative accumulates max(2G - sq_j) over other-class tiles,
    hardest-positive accumulates min(2G - sq_j) over the own-class tile's
    real rows.  GPSIMD partition_all_reduce(max) folds the 128 j-partitions.
  - hn2 = sq_a - AR(acc_hn), hp2 = sq_a + AR(-acc_hp); sqrt (+1 Newton step),
    loss = relu(hp - hn + 1) on chip; host masks dummy anchors and averages.
"""

import numpy as np

import concourse.bass as bass
import concourse.bacc as bacc
import concourse.tile as tile
from concourse import mybir
from concourse.bass_utils import run_bass_kernel_spmd
from concourse import bass_isa

B, D, NCLASS = 8192, 128, 10
SEG = 1024                 # padded rows per class
TPC = SEG // 128           # 128-row j-tiles per class = 8
NCORES = 8
BPAD = NCLASS * SEG        # 10240
NJT = BPAD // 128          # 80 j-tiles
NA = NCLASS * 128          # anchors per core = 1280
F32 = mybir.dt.float32
BF16 = mybir.dt.bfloat16
AFT = mybir.ActivationFunctionType
ALU = mybir.AluOpType
MARGIN = 1.0
DUMMY_VAL = 1000.0


def build_nc(R):
    """R: real row count per class (512 < R[k] <= SEG)."""
    nc = bacc.Bacc()
    ebt_d = nc.dram_tensor("ebt", [D, BPAD], F32, kind="ExternalInput")
    ant_d = nc.dram_tensor("anch_t", [D, NA], F32, kind="ExternalInput")
    epn_d = nc.dram_tensor("ep_nat", [BPAD, D], F32, kind="ExternalInput")
    out_d = nc.dram_tensor("out", [256, NA], F32, kind="ExternalOutput")

    with tile.TileContext(nc) as tc:
        with (
            tc.tile_pool(name="big", bufs=1) as big,
            tc.tile_pool(name="small", bufs=1) as small,
            tc.tile_pool(name="psum", bufs=2, space=bass.MemorySpace.PSUM) as psum,
        ):
            # ---------------- load inputs ----------------
            ebt_f = big.tile([D, BPAD], F32, tag="ebt_f")
            nc.sync.dma_start(ebt_f[:], ebt_d[:])
            an_t = big.tile([D, NA], F32, tag="an_t")
            nc.sync.dma_start(an_t[:], ant_d[:])
            epn = big.tile([128, NJT, D], F32, tag="epn")
            nc.sync.dma_start(epn[:], epn_d.rearrange("(t q) d -> q t d", q=128))

            # sqv[q, t] = ||e_{t*128+q}||^2 first (so later DVE ops cover it
            # transitively through the PE wait chain)
            nc.scalar.activation(epn[:], epn[:], AFT.Square)
            sqv = small.tile([128, NJT], F32, tag="sqv")
            nc.vector.reduce_sum(sqv[:], epn[:], axis=mybir.AxisListType.X)

            ebt_b = big.tile([D, BPAD], BF16, tag="ebt_b")
            nc.vector.tensor_copy(ebt_b[:], ebt_f[:])
            an2_b = big.tile([D, NA], BF16, tag="an2_b")
            nc.vector.tensor_scalar_mul(an2_b[:], an_t[:], 2.0)

            # accumulators: no memset -- the first touch of each region is a
            # plain tensor_scalar write (keeps every DVE op at <=1 sync wait)
            acc_hn = big.tile([128, NA], F32, tag="acc_hn")
            acc_hp = big.tile([128, NA], F32, tag="acc_hp")

            # ---------------- main loop over 80 j-tiles ----------------
            touched_hn, touched_hp = set(), set()
            for t in range(NJT):
                c, ri = t // TPC, t % TPC
                nreal = min(max(int(R[c]) - ri * 128, 0), 128)
                g = psum.tile([128, NA], F32, tag="ps")
                for h, w in ((0, 512), (512, 512), (1024, 256)):
                    nc.tensor.matmul(g[:, h:h + w],
                                     ebt_b[:, t * 128:(t + 1) * 128],
                                     an2_b[:, h:h + w], start=True, stop=True)
                sq_t = sqv[:, t:t + 1]

                def upd(acc, rows, lo, hi, op1, touched, key):
                    seg, segs = None, []
                    for k in range(lo // 128, hi // 128):
                        if (key, k) in touched:
                            if seg and seg[2]:
                                seg = (seg[0], k + 1, True)
                                segs[-1] = seg
                            elif seg is None or not seg[2]:
                                seg = (k, k + 1, True)
                                segs.append(seg)
                            else:
                                seg = (k, k + 1, True)
                                segs.append(seg)
                        else:
                            if seg and not seg[2]:
                                seg = (seg[0], k + 1, False)
                                segs[-1] = seg
                            else:
                                seg = (k, k + 1, False)
                                segs.append(seg)
                            touched.add((key, k))
                    for a, b, is_acc in segs:
                        sl = slice(a * 128, b * 128)
                        if is_acc:
                            nc.vector.scalar_tensor_tensor(
                                acc[rows, sl], g[rows, sl], sq_t[rows],
                                acc[rows, sl], op0=ALU.subtract, op1=op1)
                        else:
                            nc.vector.tensor_scalar(
                                acc[rows, sl], g[rows, sl], sq_t[rows], None,
                                op0=ALU.subtract)

                # hardest-negative: all anchor columns except own class c
                if c > 0:
                    upd(acc_hn, slice(0, 128), 0, c * 128, ALU.max, touched_hn, 0)
                if c < NCLASS - 1:
                    upd(acc_hn, slice(0, 128), (c + 1) * 128, NA, ALU.max,
                        touched_hn, 0)
                # hardest-positive: own-class columns, real j rows only
                if nreal > 0:
                    upd(acc_hp, slice(0, nreal), c * 128, (c + 1) * 128, ALU.min,
                        touched_hp, 0)

            # ---------------- ship both accumulators; host folds ----------
            nc.sync.dma_start(out_d[0:128, :], acc_hn[:])
            nc.sync.dma_start(out_d[128:256, :], acc_hp[:])
    nc.compile()
    return nc


def prepare(embeddings, labels):
    emb = np.ascontiguousarray(np.asarray(embeddings, dtype=np.float32))
    lab = np.asarray(labels).astype(np.int64).ravel()
    assert emb.shape == (B, D)
    order = np.argsort(lab, kind="stable")
    es = emb[order]
    counts = np.bincount(lab, minlength=NCLASS)
    assert counts.max() <= SEG and counts.min() > 512, counts
    ep = np.zeros((BPAD, D), np.float32)
    ep[:, 0] = DUMMY_VAL
    ofs = np.concatenate([[0], np.cumsum(counts)])
    for c in range(NCLASS):
        ep[c * SEG: c * SEG + counts[c]] = es[ofs[c]:ofs[c + 1]]
    ebt = np.ascontiguousarray(ep.T)
    in_maps = []
    for i in range(NCORES):
        rows = [
            ep[k * SEG + ((i + k) % TPC) * 128: k * SEG + ((i + k) % TPC + 1) * 128]
            for k in range(NCLASS)
        ]
        anch = np.ascontiguousarray(np.concatenate(rows, 0))
        in_maps.append({
            "ebt": ebt,
            "anch_t": np.ascontiguousarray(anch.T),
            "ep_nat": ep,
        })
    return in_maps, counts


def combine(results, counts, in_maps):
    total = 0.0
    for i in range(NCORES):
        o = np.asarray(results[i]["out"], np.float32)
        hn_m = np.max(o[0:128], axis=0)    # max over j-partials of (2G - sq_j)
        hp_m = np.min(o[128:256], axis=0)  # min over own-class partials
        anch = in_maps[i]["anch_t"].T.astype(np.float32)
        sq_a = np.sum(anch * anch, axis=1)
        hn = np.sqrt(np.maximum(sq_a - hn_m, 0.0))
        hp = np.sqrt(np.maximum(sq_a - hp_m, 0.0))
        li = np.maximum(hp - hn + np.float32(MARGIN), 0.0)
        for k in range(NCLASS):
            t = (i + k) % TPC
            nreal = int(np.clip(int(counts[k]) - t * 128, 0, 128))
            if nreal > 0:
                total += float(np.sum(li[k * 128: k * 128 + nreal].astype(np.float64)))
    return np.asarray(total / B, dtype=np.float32)


def kernel(embeddings, labels, _trace=False, _tmpdir=None):
    in_maps, counts = prepare(embeddings, labels)
    nc = build_nc(list(counts))
    res = run_bass_kernel_spmd(
        nc, in_maps, list(range(NCORES)), trace=_trace, tmpdir=_tmpdir
    )
    out = combine(res.results, counts, in_maps)
    if _trace:
        return out, res
    return out
